# revision 1
# baseline (speedup 1.0000x reference)
"""BlockCirculantLinear kernel for 8x TRN2 NeuronCores — FFT-domain einsum.

Math: out = (x*D) @ M with M block-circulant (32x32 blocks of 128-circulants).
The reference computes per-block circular correlation in the FFT domain; a
dense matmul costs 2*B*4096^2 FLOPs but the frequency-domain einsum
out_fft[b,o,f] = sum_j Xf[b,j,f] * conj(Wf)[o,j,f] costs ~32x less. Host
does the cheap O(B d log b) rfft/irfft + packing; the device does the
einsum — where the FLOPs are — as bf16 matmuls.

Packing: rfft of a real 128-signal = 65 bins; bins 1..63 complex, 0/64
real. Exactly 128 real planes per block: R0..R63, I0..I63 with the I0
slot carrying R64. Planes are grouped 4 bins per 128-partition tile
(p = fi*32 + j) and the per-bin 32x32 complex multiply becomes 4 real
matmuls psR = A.XR + B.XI, psI = C.XR + D.XI with A=Re(V), B=-Im(V),
C=Im(V), D=Re(V), V = conj(rfft(W)); the (g=0,fi=0) slot is special-
cased (B=C=0, D=Re(V64)) so psR0/psI0 carry the two real bins. The j-
contraction is only 32 deep, so the 4 bins of a group run as concurrent
32x32 quadrant matmuls via tile_position=(32fi,32fi) — weights stay
dense (0.5MB, not 2MB block-diagonal).

Batch is data-parallel across 8 cores (1024 samples each). Per-core:
16 groups x 8 accumulation steps of 4 quadrant matmuls [32,32]x[32,512]
bf16 -> f32 PSUM; psR evacuated by VectorE, psI by ScalarE, cast bf16.
I/O: 8MB in + 8MB out + 0.5MB weights, moved as 1MB two-group units
with fully contiguous 8KB partition rows (per-DMA ring turnaround is
~3.4us, so smaller units waste bandwidth). Inputs stream on the ACT
HWDGE ring; outputs ride the Sync ring early and the drained ACT ring
late, with the final group in 256KB chunks so the last completion
(which gates the exit barrier) lands early. Dummy matmuls pre-warm the
PE clock-gate (HAM) while the first input streams in, and 2 keep-warm
fillers per group stop mid-stream re-throttles when an input DMA runs
late. The wall: ~17MB over a ~450-550GB/s R+W HBM envelope (~35us),
~27us of warm PE inside it, plus ~7us NEFF startup and ~10us Tile
exit-barrier/IRAM-fetch epilogue. Measured 65-66us (baseline dense
fp32r matmul: 528us).

Measured end-to-end relative error ~3e-3 (bf16 rounding; fp8 inputs
fail the 2e-2 gate at 2.7e-2).
"""

import numpy as np
import ml_dtypes

B_TOTAL = 8192
D_IN = 4096
D_OUT = 4096
BLK = 128
K_IN = D_IN // BLK    # 32
K_OUT = D_OUT // BLK  # 32
N_CORES = 8
B_SHARD = B_TOTAL // N_CORES  # 1024
NB = BLK // 2 + 1     # 65 rfft bins
G = 16                # groups of 4 packed bins (64 plane-pairs)
NP = G // 2           # group pairs = DMA units of 1MB
MM_FREE = 512         # moving free dim per matmul (one PSUM bank)

_compiled = None


def _build_module():
    import concourse.bass as bass
    import concourse.tile as tile
    from concourse import bacc, mybir

    nc = bacc.Bacc("TRN2", target_bir_lowering=False, debug=False)

    bf = mybir.dt.bfloat16
    f32 = mybir.dt.float32

    # xf[pair, p, gi, c, m] flattened to [pair, p, 4096]: contiguous 8KB rows
    xf = nc.dram_tensor("xf", [NP, 128, 2, 2, B_SHARD], bf, kind="ExternalInput")
    # wt[p, g, wk, q]: dense per-quadrant lhsT blocks, wk in (A, B, C, D)
    wt = nc.dram_tensor("wt", [128, G, 4, 32], bf, kind="ExternalInput")
    # yf[pair, p, gi, c, m]: c=0 psR, c=1 psI; p = fi*32+o
    yf = nc.dram_tensor("yf", [NP, 128, 2, 2, B_SHARD], bf, kind="ExternalOutput")

    PAIR_ELEMS = 128 * 4 * B_SHARD

    with tile.TileContext(nc) as tc:
        with (
            tc.tile_pool(name="sb", bufs=1) as spool,
            tc.tile_pool(name="psum", bufs=2, space="PSUM") as ppool,
        ):
            w = spool.tile([128, G, 4, 32], bf, name="wt")
            nc.sync.dma_start(w[:], wt[:])

            scratch = spool.tile([128, MM_FREE], bf, name="scratch")
            nc.vector.memset(scratch[:], 0.0)

            # all input DMAs queued up-front, alternating HWDGE rings, so
            # they drain ahead of the (later-queued) output DMAs
            xts = []
            for pr in range(NP):
                xt = spool.tile(
                    [128, 2, 2, B_SHARD], bf, tag="xt", name=f"xt{pr}", bufs=NP
                )
                # 4/4 ring split, odd pairs on sync: both rows pull reads in
                # parallel, and the LAST input pair sits behind only 4.5MB on
                # the sync row (vs 8.5MB of reads+writes on the ACT row), so
                # it lands ~27us — before the PE needs it — instead of ~40us
                eng = nc.sync if pr % 2 == 1 else nc.scalar
                eng.dma_start(
                    xt[:],
                    bass.AP(
                        xf, pr * PAIR_ELEMS, [[4 * B_SHARD, 128], [1, 4 * B_SHARD]]
                    ),
                )
                xts.append(xt)

            ot = None
            for g in range(G):
                pr, gi = g // 2, g % 2
                xt = xts[pr]
                psRs = [
                    ppool.tile([128, MM_FREE], f32, tag=f"psR{mc}", name=f"psR{mc}_{g}")
                    for mc in range(2)
                ]
                psIs = [
                    ppool.tile([128, MM_FREE], f32, tag=f"psI{mc}", name=f"psI{mc}_{g}")
                    for mc in range(2)
                ]
                psR, psI = psRs[0], psIs[0]
                if g == 0:
                    # HAM pre-warm: keep the PE busy on garbage matmuls while
                    # the first input streams in, so real matmuls run at 2.4
                    # GHz from the start (the clock gate needs ~3.4us of
                    # sustained activity; results overwritten by start=True)
                    for k in range(24):
                        nc.tensor.matmul(
                            (psR if k % 2 == 0 else psI)[:, 0:MM_FREE],
                            lhsT=scratch[:, 0:128],
                            rhs=scratch[:],
                            start=True,
                            stop=True,
                        )
                elif g < G - 2:
                    # keep-warm filler: if the PE is waiting on an input DMA
                    # here, these stop the clock-gate from re-throttling; the
                    # real first matmul's start=True wipes the garbage
                    for k in range(2):
                        nc.tensor.matmul(
                            (psR if k % 2 == 0 else psI)[:, 0:MM_FREE],
                            lhsT=scratch[:, 0:128],
                            rhs=scratch[:],
                            start=True,
                            stop=True,
                        )
                for mc in range(B_SHARD // MM_FREE):
                    s = slice(mc * MM_FREE, (mc + 1) * MM_FREE)
                    # psR = A.XR + B.XI on sub-arrays (fi,fi); psI = C.XR +
                    # D.XI on sub-arrays (fi,(fi+1)%4). Consecutive steps hit
                    # DISJOINT sub-arrays, so each step's LDWEIGHTS overlaps
                    # the previous step's matmul instead of waiting for its
                    # drain (same-row_grp LDW cannot be pulled ahead). psI
                    # lands column-rotated in PSUM; the host unpack un-rotates.
                    for wk, c, ps, rot, st, sp in (
                        (0, 0, psRs[mc], 0, True, False),
                        (2, 0, psIs[mc], 1, True, False),
                        (1, 1, psRs[mc], 0, False, True),
                        (3, 1, psIs[mc], 1, False, True),
                    ):
                        for fi in range(4):
                            q = slice(fi * 32, (fi + 1) * 32)
                            fo = (fi + rot) % 4
                            nc.tensor.matmul(
                                ps[fo * 32 : (fo + 1) * 32, :],
                                lhsT=w[q, g, wk, :],
                                rhs=xt[q, gi, c, s],
                                start=st,
                                stop=sp,
                                tile_position=(fi * 32, fo * 32),
                            )

                if pr == NP - 1:
                    # separate per-group tiles for the last pair: the split
                    # final DMAs otherwise inherit a tile-granular dependency
                    # on BOTH groups' copies and issue ~3.5us late
                    ot = spool.tile(
                        [128, 1, 2, B_SHARD], bf, tag="otl", name=f"otl{gi}", bufs=2
                    )
                    oslice = ot[:, 0, :, :]
                elif gi == 0:
                    ot = spool.tile(
                        [128, 2, 2, B_SHARD], bf, tag="ot", name=f"ot{pr}", bufs=6
                    )
                    oslice = ot[:, gi, :, :]
                else:
                    oslice = ot[:, gi, :, :]
                for mc in range(2):
                    s = slice(mc * MM_FREE, (mc + 1) * MM_FREE)
                    nc.vector.tensor_copy(oslice[:, 0, s], psRs[mc][:])
                    nc.scalar.copy(oslice[:, 1, s], psIs[mc][:])
                # output DMAs: 1MB pair units (per-DMA ring turnaround ~3.4us
                # ~ pair production rate, so bigger units waste less of it).
                # Early pairs ride the idle Sync ring; late pairs the ACT
                # ring, whose input FIFO has drained by the time they're
                # ready. The last pair goes as fine-grained chunks spread
                # over both rings so the final completion (which gates the
                # epilogue) lands as early as possible.
                if pr < NP - 1:
                    if gi == 1:
                        # three write rows, each fed when it has spare
                        # capacity: sync is free from the start, the SWDGE
                        # (gpsimd) row takes the middle pairs, and the ACT
                        # row picks up once its input FIFO drains (~23us)
                        oeng = {0: nc.sync, 1: nc.scalar, 2: nc.sync,
                                3: nc.gpsimd, 4: nc.gpsimd, 5: nc.scalar,
                                6: nc.scalar}[pr]
                        oeng.dma_start(
                            bass.AP(
                                yf,
                                pr * PAIR_ELEMS,
                                [[4 * B_SHARD, 128], [1, 4 * B_SHARD]],
                            ),
                            ot[:],
                        )
                elif gi == 0:
                    nc.sync.dma_start(
                        bass.AP(
                            yf, pr * PAIR_ELEMS, [[4 * B_SHARD, 128], [1, 2 * B_SHARD]]
                        ),
                        oslice[:],
                    )
                else:
                    # last group: 4x128KB chunks, each gated by exactly one
                    # half-group copy, spread over all three write rows; the
                    # final chunk rides the empty sync row so its completion
                    # receipt (which gates the exit barrier) fires ~2.5us
                    # after the last copy instead of ~8us
                    for (mc, c), oeng in (
                        ((0, 0), nc.gpsimd),
                        ((0, 1), nc.gpsimd),
                        ((1, 0), nc.scalar),
                        ((1, 1), nc.sync),
                    ):
                        oeng.dma_start(
                            bass.AP(
                                yf,
                                pr * PAIR_ELEMS + (2 + c) * B_SHARD + mc * MM_FREE,
                                [[4 * B_SHARD, 128], [1, MM_FREE]],
                            ),
                            oslice[:, c, mc * MM_FREE : (mc + 1) * MM_FREE],
                        )

    nc.compile()
    return nc


def _get_module():
    global _compiled
    if _compiled is None:
        _compiled = _build_module()
    return _compiled


def kernel(x: np.ndarray, W: np.ndarray, D_bernoulli: np.ndarray) -> np.ndarray:
    from concourse.bass_utils import run_bass_kernel_spmd

    bf16 = ml_dtypes.bfloat16
    x = np.asarray(x, dtype=np.float32)
    W = np.asarray(W, dtype=np.float32)
    D = np.asarray(D_bernoulli, dtype=np.float32)

    # --- host: forward rfft of (x*D) blocks, pack 64 plane-pair groups ---
    xd = (x * D[None, :]).reshape(B_TOTAL, K_IN, BLK)
    Xf = np.fft.rfft(xd, axis=-1)                 # [B, 32, 65]
    Xr = np.ascontiguousarray(Xf.real.transpose(2, 1, 0))  # [65, 32, B]
    Xi = np.ascontiguousarray(Xf.imag.transpose(2, 1, 0))
    XR = Xr[:64]                                  # [64, 32, B]
    XI = Xi[:64].copy()
    XI[0] = Xr[64]                                # R64 rides in the I0 slot
    # xf_all[pair, p, gi, c, m_global]
    xg = np.empty((G, 128, 2, B_TOTAL), dtype=bf16)
    xg[:, :, 0, :] = XR.reshape(G, 128, B_TOTAL)
    xg[:, :, 1, :] = XI.reshape(G, 128, B_TOTAL)
    xf_all = np.ascontiguousarray(
        xg.reshape(NP, 2, 128, 2, B_TOTAL).transpose(0, 2, 1, 3, 4)
    )

    # --- host: weights -> dense quadrant lhsT blocks [p, G, wk, 32] ---
    Vf = np.conj(np.fft.rfft(W, axis=-1))         # [o, j, 65]
    VR = Vf.real.transpose(2, 1, 0)               # [65, j, o]
    VI = Vf.imag.transpose(2, 1, 0)
    A = VR[:64].copy()
    Bm = (-VI[:64]).copy()
    C = VI[:64].copy()
    Dm = VR[:64].copy()
    Bm[0] = 0.0                                   # bin-0/64 real-only slots
    C[0] = 0.0
    Dm[0] = VR[64]
    Wd = np.stack((A, Bm, C, Dm), axis=1)         # [64, 4, j32, o32]
    # -> [p = fi*32+j, g, wk, o]
    wt_host = np.ascontiguousarray(
        Wd.reshape(G, 4, 4, K_IN, K_OUT).transpose(1, 3, 0, 2, 4).reshape(128, G, 4, K_OUT)
    ).astype(bf16)

    in_maps = []
    for c in range(N_CORES):
        sl = slice(c * B_SHARD, (c + 1) * B_SHARD)
        in_maps.append({"xf": np.ascontiguousarray(xf_all[:, :, :, :, sl]), "wt": wt_host})

    nc = _get_module()
    res = run_bass_kernel_spmd(nc, in_maps, core_ids=list(range(N_CORES)))

    # --- host: unpack spectra, irfft, reassemble ---
    out = np.empty((B_TOTAL, D_OUT), dtype=np.float32)
    for c in range(N_CORES):
        y = np.asarray(res.results[c]["yf"], dtype=np.float32)  # [NP,128,2,2,m]
        # -> [g, block, o, ch, m]; psI quads land column-rotated by +1 block
        # (device uses disjoint sub-arrays for psR/psI) — roll undoes it
        yb = y.transpose(0, 2, 1, 3, 4).reshape(G, 4, K_OUT, 2, B_SHARD)
        psR = yb[:, :, :, 0, :].reshape(64, K_OUT, B_SHARD)
        psI = np.roll(yb[:, :, :, 1, :], -1, axis=1).reshape(64, K_OUT, B_SHARD)
        Yf = np.zeros((B_SHARD, K_OUT, NB), dtype=np.complex64)
        Yf[:, :, :64] = (psR + 1j * psI).transpose(2, 1, 0)
        Yf[:, :, 0] = psR[0].T
        Yf[:, :, 64] = psI[0].T
        ob = np.fft.irfft(Yf, n=BLK, axis=-1)     # [m, 32, 128]
        out[c * B_SHARD : (c + 1) * B_SHARD] = ob.reshape(B_SHARD, D_OUT)
    return out



# revision 2
# speedup vs baseline: 1.1952x; 1.1952x over previous
"""BlockCirculantLinear kernel for 8x TRN2 NeuronCores — FFT-domain einsum, v2.

Math: out = (x*D) @ M with M block-circulant (32x32 grid of 128-circulants).
Host does the cheap O(B d log b) rfft/irfft + packing; the device does the
frequency-domain einsum out_f = X_f @ V_f (a 32x32 complex matmul per bin).

v2 design (vs the 66us v1):
- Each bin's complex matmul is ONE dense 64x64 real matmul via the
  [[Re, Im], [-Im, Re]] block form: rhs = [XR; XI] (64 partitions),
  out = [YR; YI]. Four bins run concurrently on the four 64x64 quadrants
  of the PE array via tile_position (a/b on rows 0-63, c/d on rows
  64-127; a/c write psum cols 0-63 of two banks, b/d cols 64-127).
  128 MMs + 128 LDWs total vs v1's 560 MMs + 562 32-col LDWs (the v1
  Tensor queue was 92% busy, mostly LDWEIGHTS).
- Input spectra ship as fp8 E3M4 (4 mantissa bits, max 15.5; spectra are
  scaled by 14/max on host, weights absorb 1/s). Mixed-dtype matmul
  (bf16 lhsT x fp8e3 rhs) keeps the weights full precision. End-to-end
  rel err 1.34e-2 (vs 3.1e-3 all-bf16, gate 2e-2); e4m3 fails at 2.8e-2.
  Input HBM traffic halves: 8MB -> 4MB per core.
- DMA: reads-first FIFO on the Sync HWDGE ring (4x 1MB input units +
  7x 1MB output pairs queue behind them), weights + final chunks on the
  scalar/ACT ring. No gpsimd/SWDGE. Total 12.5MB/core at the ~360GB/s
  per-NC HBM cap ~= 35us DMA window, vs 17.3MB/48us in v1.
- HAM pre-warm garbage MMs cover the first input unit's flight time.
"""

import numpy as np
import ml_dtypes

B_TOTAL = 8192
D_IN = 4096
D_OUT = 4096
BLK = 128
K_IN = D_IN // BLK    # 32
K_OUT = D_OUT // BLK  # 32
N_CORES = 8
B_SHARD = B_TOTAL // N_CORES  # 1024
NG = 16               # groups of 4 bins (64 plane-pairs)
NU = 4                # input DMA units (4 groups = 1MB each)
NPO = 8               # output DMA pairs (2 groups = 1MB each)
MM_FREE = 512         # moving free dim per matmul (one PSUM bank)
XSCALE_TGT = 14.0     # fp8 e3m4 max normal is 15.5

_compiled = None


def _build_module():
    import concourse.bass as bass
    import concourse.tile as tile
    from concourse import bacc, mybir

    nc = bacc.Bacc("TRN2", target_bir_lowering=False, debug=False)

    bf = mybir.dt.bfloat16
    f8 = mybir.dt.float8e3
    f32 = mybir.dt.float32

    # xq[unit, p, gi, u, m]: p = 64*half + 32*comp + j; bin = 4g + 2*half + u
    xq = nc.dram_tensor("xq", [NU, 128, 4, 2, B_SHARD], f8, kind="ExternalInput")
    # wt[p, g, u, oc]: p = 64*half + jc; lhsT of the 64x64 bin matrix
    wt = nc.dram_tensor("wt", [128, NG, 2, 64], bf, kind="ExternalInput")
    # yf[pair, p, gj, bank, m]: p = 64*outhalf + 32*comp + o; bin = 4g + 2*bank + outhalf
    yf = nc.dram_tensor("yf", [NPO, 128, 2, 2, B_SHARD], bf, kind="ExternalOutput")

    U_ELEMS = 128 * 4 * 2 * B_SHARD   # 1MB fp8 per input unit
    P_ELEMS = 128 * 2 * 2 * B_SHARD   # per output pair (x2B = 1MB)

    with tile.TileContext(nc) as tc:
        with (
            tc.tile_pool(name="sb", bufs=1) as spool,
            tc.tile_pool(name="psum", bufs=2, space="PSUM") as ppool,
        ):
            w = spool.tile([128, NG, 2, 64], bf, name="wt")
            # weights ride the ACT ring, in parallel with the input stream
            nc.scalar.dma_start(w[:], wt[:])

            scratch = spool.tile([128, MM_FREE], bf, name="scratch")
            nc.vector.memset(scratch[:], 0.0)

            # all input units queued up-front on the Sync/SP ring; output
            # DMAs are issued later on the same ring, so the FIFO gives
            # reads absolute priority (optimal makespan: the last input
            # gates the last compute which gates the tail of the writes)
            xts = []
            for ui in range(NU):
                xt = spool.tile(
                    [128, 4, 2, B_SHARD], f8, tag="xt", name=f"xt{ui}", bufs=NU
                )
                nc.sync.dma_start(
                    xt[:],
                    bass.AP(xq, ui * U_ELEMS, [[4 * 2 * B_SHARD, 128], [1, 4 * 2 * B_SHARD]]),
                )
                xts.append(xt)

            ot = None
            for g in range(NG):
                ui, gi = g // 4, g % 4
                xt = xts[ui]
                # ps[0/1] = bank A (bins a,b) mc 0/1; ps[2/3] = bank B (c,d)
                ps = [
                    ppool.tile([128, MM_FREE], f32, tag=f"ps{k}", name=f"ps{k}_{g}")
                    for k in range(4)
                ]
                if g == 0:
                    # HAM pre-warm: garbage matmuls while the first input
                    # unit streams in, so real matmuls run at 2.4 GHz
                    # (~3.4us of sustained PE activity trips the un-throttle;
                    # results are wiped by the real MMs' start=True)
                    for k in range(14):
                        nc.tensor.matmul(
                            ps[k % 4][:, :],
                            lhsT=scratch[:, 0:128],
                            rhs=scratch[:],
                            start=True,
                            stop=True,
                        )
                elif g % 4 != 0 and g < NG - 2:
                    # keep-warm filler at group boundaries inside a landed
                    # unit; insurance against input-DMA jitter re-throttling
                    # the clock gate mid-stream
                    nc.tensor.matmul(
                        ps[0][:, :],
                        lhsT=scratch[:, 0:128],
                        rhs=scratch[:],
                        start=True,
                        stop=True,
                    )
                for mc in range(2):
                    s = slice(mc * MM_FREE, (mc + 1) * MM_FREE)
                    for half, u, pk, tp in (
                        (0, 0, 0, (0, 0)),
                        (0, 1, 0, (0, 64)),
                        (1, 0, 2, (64, 0)),
                        (1, 1, 2, (64, 64)),
                    ):
                        rows = slice(64 * half, 64 * half + 64)
                        cols = slice(tp[1], tp[1] + 64)
                        nc.tensor.matmul(
                            ps[pk + mc][cols, :],
                            lhsT=w[rows, g, u, :],
                            rhs=xt[rows, gi, u, s],
                            start=True,
                            stop=True,
                            tile_position=tp,
                        )

                pr, gj = g // 2, g % 2
                if g >= NG - 2:
                    # separate per-group tiles for the final pair so the
                    # split last DMAs gate on one group's copies only
                    ot = spool.tile(
                        [128, 1, 2, B_SHARD], bf, tag="otl", name=f"otl{gj}", bufs=2
                    )
                    osl = ot[:, 0, :, :]
                elif gj == 0:
                    ot = spool.tile(
                        [128, 2, 2, B_SHARD], bf, tag="ot", name=f"ot{pr}", bufs=NPO - 1
                    )
                    osl = ot[:, gj, :, :]
                else:
                    osl = ot[:, gj, :, :]
                # evacuate PSUM: vector takes bank A, scalar takes bank B
                nc.vector.tensor_copy(osl[:, 0, 0:MM_FREE], ps[0][:])
                nc.vector.tensor_copy(osl[:, 0, MM_FREE:2 * MM_FREE], ps[1][:])
                nc.scalar.copy(osl[:, 1, 0:MM_FREE], ps[2][:])
                nc.scalar.copy(osl[:, 1, MM_FREE:2 * MM_FREE], ps[3][:])

                if pr < NPO - 1:
                    if gj == 1:
                        nc.sync.dma_start(
                            bass.AP(
                                yf,
                                pr * P_ELEMS,
                                [[2 * 2 * B_SHARD, 128], [1, 2 * 2 * B_SHARD]],
                            ),
                            ot[:],
                        )
                else:
                    # final pair: 4x 256KB chunks, each gated on one engine's
                    # copies of one group, spread over both HWDGE rings so
                    # the exit-gating completions land as early as possible
                    for bank, oeng in ((0, nc.sync), (1, nc.scalar)):
                        oeng.dma_start(
                            bass.AP(
                                yf,
                                pr * P_ELEMS + gj * 2 * B_SHARD + bank * B_SHARD,
                                [[2 * 2 * B_SHARD, 128], [1, B_SHARD]],
                            ),
                            osl[:, bank, :],
                        )

    nc.compile()
    return nc


def _get_module():
    global _compiled
    if _compiled is None:
        _compiled = _build_module()
    return _compiled


def kernel(x: np.ndarray, W: np.ndarray, D_bernoulli: np.ndarray) -> np.ndarray:
    from concourse.bass_utils import run_bass_kernel_spmd

    bf16 = ml_dtypes.bfloat16
    e3m4 = ml_dtypes.float8_e3m4
    x = np.asarray(x, dtype=np.float32)
    W = np.asarray(W, dtype=np.float32)
    D = np.asarray(D_bernoulli, dtype=np.float32)

    # --- host: forward rfft of (x*D) blocks ---
    xd = (x * D[None, :]).reshape(B_TOTAL, K_IN, BLK)
    Xf = np.fft.rfft(xd, axis=-1)                 # [B, 32, 65]
    Xr = np.ascontiguousarray(Xf.real.transpose(2, 1, 0)).astype(np.float32)  # [65, 32, B]
    Xi = np.ascontiguousarray(Xf.imag.transpose(2, 1, 0)).astype(np.float32)
    XR = Xr[:64]                                  # [64, 32, B]
    XI = Xi[:64].copy()
    XI[0] = Xr[64]                                # R64 rides in the I0 slot

    # fp8 e3m4 scale: map the max |spectrum| to ~14 (max normal 15.5);
    # the weights absorb 1/s so the product is unchanged
    s = XSCALE_TGT / max(np.abs(XR).max(), np.abs(XI).max())

    # xq_all[un, p= (half,comp,j), gi, u, m]; bin = 4*(4*un+gi) + 2*half + u
    Z = np.stack([XR * s, XI * s], axis=1)        # [64, 2(comp), 32, B]
    Z3 = Z.reshape(NU, 4, 2, 2, 2, K_IN, B_TOTAL)  # [un, gi, half, u, comp, j, B]
    xq_all = np.ascontiguousarray(
        Z3.transpose(0, 2, 4, 5, 1, 3, 6).reshape(NU, 128, 4, 2, B_TOTAL)
    ).astype(e3m4)

    # --- host: weights -> 64x64 bin matrices [[A,C],[B,D]] (lhsT) ---
    Vf = np.conj(np.fft.rfft(W, axis=-1)) / s     # [o, j, 65]
    VR = np.ascontiguousarray(Vf.real.transpose(2, 1, 0)).astype(np.float32)  # [65, j, o]
    VI = np.ascontiguousarray(Vf.imag.transpose(2, 1, 0)).astype(np.float32)
    M2 = np.empty((64, 64, 64), dtype=np.float32)  # [bin, jc, oc]
    M2[:, :K_IN, :K_OUT] = VR[:64]                 # A  (YR += A.XR)
    M2[:, :K_IN, K_OUT:] = VI[:64]                 # C  (YI += C.XR)
    M2[:, K_IN:, :K_OUT] = -VI[:64]                # B  (YR += B.XI)
    M2[:, K_IN:, K_OUT:] = VR[:64]                 # D  (YI += D.XI)
    M2[0, :K_IN, K_OUT:] = 0.0                     # bin 0/64 are real-only
    M2[0, K_IN:, :K_OUT] = 0.0
    M2[0, K_IN:, K_OUT:] = VR[64]                  # R64 channel in the I0 slot
    wt_host = np.ascontiguousarray(
        M2.reshape(NG, 2, 2, 64, 64).transpose(1, 3, 0, 2, 4).reshape(128, NG, 2, 64)
    ).astype(bf16)

    in_maps = []
    for c in range(N_CORES):
        sl = slice(c * B_SHARD, (c + 1) * B_SHARD)
        in_maps.append(
            {"xq": np.ascontiguousarray(xq_all[:, :, :, :, sl]), "wt": wt_host}
        )

    nc = _get_module()
    res = run_bass_kernel_spmd(nc, in_maps, core_ids=list(range(N_CORES)))

    # --- host: unpack spectra, irfft, reassemble ---
    out = np.empty((B_TOTAL, D_OUT), dtype=np.float32)
    NB = BLK // 2 + 1
    for c in range(N_CORES):
        y = np.asarray(res.results[c]["yf"], dtype=np.float32)  # [NPO,128,2,2,m]
        # y[pr, 64*outhalf+32*comp+o, gj, bank, m]; bin = 4*(2pr+gj)+2*bank+outhalf
        yb = y.reshape(NPO, 2, 2, K_OUT, 2, 2, B_SHARD)  # [pr, outhalf, comp, o, gj, bank, m]
        PS = yb.transpose(0, 4, 5, 1, 2, 3, 6).reshape(64, 2, K_OUT, B_SHARD)
        psR, psI = PS[:, 0], PS[:, 1]                    # [64, o, m]
        Yf = np.zeros((B_SHARD, K_OUT, NB), dtype=np.complex64)
        Yf[:, :, :64] = (psR + 1j * psI).transpose(2, 1, 0)
        Yf[:, :, 0] = psR[0].T
        Yf[:, :, 64] = psI[0].T
        ob = np.fft.irfft(Yf, n=BLK, axis=-1)            # [m, 32, 128]
        out[c * B_SHARD : (c + 1) * B_SHARD] = ob.reshape(B_SHARD, D_OUT)
    return out


# revision 3
# speedup vs baseline: 1.3144x; 1.0998x over previous
"""BlockCirculantLinear kernel for 8x TRN2 NeuronCores — FFT-domain einsum, v3.

Math: out = (x*D) @ M with M block-circulant (32x32 grid of 128-circulants).
Host does the cheap O(B d log b) rfft/irfft + packing; the device does the
frequency-domain einsum out_f = X_f @ V_f (a 32x32 complex matmul per bin).

Device kernel (per core, 1/8 of the batch = 1024 rows):
- Each bin's complex matmul is ONE dense 64x64 real matmul via the
  [[Re, Im], [-Im, Re]] block form: rhs = [XR; XI] (64 partitions),
  out = [YR; YI]. Four bins run concurrently on the four 64x64 quadrants
  of the PE array via tile_position; 128 MMs total.
- Input spectra ship as fp8 E3M4 (4 mantissa bits), scaled by 14/max
  on host; mixed-dtype matmul (bf16 lhsT x fp8e3 rhs) keeps weights
  full precision. Input HBM: 8MB -> 4.2MB per core.
- Output: the 32 lowest-energy bins (ranked by a per-bin energy proxy)
  are routed to the partition-half-1 slots and evacuated as fp8 E3M4
  (psum -> SBUF cast on the scalar engine); the 32 high-energy bins
  stay bf16 (vector engine). Per-bin scales, folded into the weights
  with a hard Cauchy-Schwarz bound (|psum| <= 12.9 < 15.5 max normal,
  overflow impossible), are divided back out on the host. Output HBM:
  8MB -> 6.3MB per core. End-to-end rel err ~1.6e-2 (gate 2e-2;
  all-bf16 is 3.1e-3, and all-fp8-out would be 1.92e-2 - too tight).
- PSUM is organized as 2-bank tiles [128, 2, 512] so each group needs
  one [128,1024] evacuation copy per engine (the v2 per-bank copies at
  ~0.7us each made the copy pipeline the critical path).
- DMA: strict FIFO on the Sync/SP HWDGE ring only: weights, 4x 1MB
  input units, then the output DMAs — reads get absolute priority,
  which is the makespan-optimal schedule at the ~360GB/s per-NC HBM
  cap. (v2 put the weights on the ACT ring, which started ~4us late
  and stalled the first real matmuls to 17.7us, HAM-cold.) Total
  ~11MB/core -> ~31us DMA window + ~9us NRT preamble + ~8us postamble.
- HAM pre-warm garbage MMs bridge until the first input unit lands.
"""

import numpy as np
import ml_dtypes

B_TOTAL = 8192
D_IN = 4096
D_OUT = 4096
BLK = 128
K_IN = D_IN // BLK    # 32
K_OUT = D_OUT // BLK  # 32
N_CORES = 8
B_SHARD = B_TOTAL // N_CORES  # 1024
NG = 16               # groups of 4 bins (64 plane-pairs)
NU = 4                # input DMA units (4 groups = 1MB each)
NPO = 8               # output DMA pairs (2 groups each)
MM_FREE = 512         # moving free dim per matmul (one PSUM bank)
XSCALE_TGT = 14.0     # fp8 e3m4 max normal is 15.5
YSCALE_TGT = 12.9

_compiled = None


def _build_module():
    import concourse.bass as bass
    import concourse.tile as tile
    from concourse import bacc, mybir

    nc = bacc.Bacc("TRN2", target_bir_lowering=False, debug=False)

    bf = mybir.dt.bfloat16
    f8 = mybir.dt.float8e3
    f32 = mybir.dt.float32

    # xq[unit, p, gi, u, m]: p = 64*half + 32*comp + j; slot = (g, half, u)
    xq = nc.dram_tensor("xq", [NU, 128, 4, 2, B_SHARD], f8, kind="ExternalInput")
    # wt[p, g, u, oc]: p = 64*half + jc; lhsT of the 64x64 bin matrix
    wt = nc.dram_tensor("wt", [128, NG, 2, 64], bf, kind="ExternalInput")
    # ybf[pair, p, gj, mc, m']: bank A (input-half-0 bins), bf16
    ybf = nc.dram_tensor("ybf", [NPO, 128, 2, 2, MM_FREE], bf, kind="ExternalOutput")
    # yq8[pair, p, gj, mc, m']: bank B (input-half-1 bins), fp8 e3m4
    yq8 = nc.dram_tensor("yq8", [NPO, 128, 2, 2, MM_FREE], f8, kind="ExternalOutput")

    U_ELEMS = 128 * 4 * 2 * B_SHARD   # 1MB fp8 per input unit
    PO_ELEMS = 128 * 2 * 2 * MM_FREE  # per output pair per tensor

    with tile.TileContext(nc) as tc:
        with (
            tc.tile_pool(name="sb", bufs=1) as spool,
            tc.tile_pool(name="psum", bufs=2, space="PSUM") as ppool,
        ):
            w = spool.tile([128, NG, 2, 64], bf, name="wt")
            # weights lead the Sync ring: land ~10.2us, before the first
            # real MMs need them (~12.6) — the ACT ring starts ~4us late
            nc.sync.dma_start(w[:], wt[:])

            scratch = spool.tile([128, MM_FREE], bf, name="scratch")
            nc.vector.memset(scratch[:], 0.0)

            xts = []
            for ui in range(NU):
                xt = spool.tile(
                    [128, 4, 2, B_SHARD], f8, tag="xt", name=f"xt{ui}", bufs=NU
                )
                nc.sync.dma_start(
                    xt[:],
                    bass.AP(xq, ui * U_ELEMS, [[4 * 2 * B_SHARD, 128], [1, 4 * 2 * B_SHARD]]),
                )
                xts.append(xt)

            otA = otB = None
            for g in range(NG):
                ui, gi = g // 4, g % 4
                xt = xts[ui]
                # psA: bank-pair for input-half-0 bins (a: cols 0-63, b: 64-127)
                # psB: bank-pair for input-half-1 bins (c, d); [128, mc, 512]
                psA = ppool.tile([128, 2, MM_FREE], f32, tag="psA", name=f"psA_{g}")
                psB = ppool.tile([128, 2, MM_FREE], f32, tag="psB", name=f"psB_{g}")
                if g == 0:
                    # HAM pre-warm: garbage matmuls while the weights and the
                    # first input unit stream in (~3.4us of sustained PE
                    # activity trips the 2.4GHz un-throttle); results are
                    # wiped by the real MMs' start=True
                    for k in range(14):
                        nc.tensor.matmul(
                            (psA if k % 2 == 0 else psB)[:, k % 2, :],
                            lhsT=scratch[:, 0:128],
                            rhs=scratch[:],
                            start=True,
                            stop=True,
                        )
                elif g % 4 != 0 and g < NG - 2:
                    # keep-warm filler; insurance against input-DMA jitter
                    nc.tensor.matmul(
                        psA[:, 0, :],
                        lhsT=scratch[:, 0:128],
                        rhs=scratch[:],
                        start=True,
                        stop=True,
                    )
                for mc in range(2):
                    s = slice(mc * MM_FREE, (mc + 1) * MM_FREE)
                    for half, u, ps, tp in (
                        (0, 0, psA, (0, 0)),
                        (0, 1, psA, (0, 64)),
                        (1, 0, psB, (64, 0)),
                        (1, 1, psB, (64, 64)),
                    ):
                        rows = slice(64 * half, 64 * half + 64)
                        cols = slice(tp[1], tp[1] + 64)
                        nc.tensor.matmul(
                            ps[cols, mc, :],
                            lhsT=w[rows, g, u, :],
                            rhs=xt[rows, gi, u, s],
                            start=True,
                            stop=True,
                            tile_position=tp,
                        )

                pr, gj = g // 2, g % 2
                if g >= NG - 2:
                    # separate per-group tiles for the final pair so the
                    # split last DMAs gate on one group's copies only
                    otA = spool.tile([128, 1, 2, MM_FREE], bf, tag="otAl", name=f"otAl{gj}", bufs=2)
                    otB = spool.tile([128, 1, 2, MM_FREE], f8, tag="otBl", name=f"otBl{gj}", bufs=2)
                    oA, oB = otA[:, 0], otB[:, 0]
                elif gj == 0:
                    otA = spool.tile([128, 2, 2, MM_FREE], bf, tag="otA", name=f"otA{pr}", bufs=NPO - 1)
                    otB = spool.tile([128, 2, 2, MM_FREE], f8, tag="otB", name=f"otB{pr}", bufs=NPO - 1)
                    oA, oB = otA[:, gj], otB[:, gj]
                else:
                    oA, oB = otA[:, gj], otB[:, gj]
                # evacuate PSUM: one [128,1024] copy per engine per group
                nc.vector.tensor_copy(oA, psA[:])
                nc.scalar.copy(oB, psB[:])

                if pr < NPO - 1:
                    if gj == 1:
                        nc.sync.dma_start(
                            bass.AP(ybf, pr * PO_ELEMS, [[2048, 128], [1, 2048]]),
                            otA[:],
                        )
                        nc.sync.dma_start(
                            bass.AP(yq8, pr * PO_ELEMS, [[2048, 128], [1, 2048]]),
                            otB[:],
                        )
                else:
                    # final pair: per-group chunks so the exit-gating
                    # completions land as early as possible
                    nc.sync.dma_start(
                        bass.AP(ybf, pr * PO_ELEMS + gj * 1024, [[2048, 128], [1, 1024]]),
                        oA,
                    )
                    nc.sync.dma_start(
                        bass.AP(yq8, pr * PO_ELEMS + gj * 1024, [[2048, 128], [1, 1024]]),
                        oB,
                    )

    nc.compile()
    return nc


def _get_module():
    global _compiled
    if _compiled is None:
        _compiled = _build_module()
    return _compiled


def kernel(x: np.ndarray, W: np.ndarray, D_bernoulli: np.ndarray) -> np.ndarray:
    from concourse.bass_utils import run_bass_kernel_spmd

    bf16 = ml_dtypes.bfloat16
    e3m4 = ml_dtypes.float8_e3m4
    x = np.asarray(x, dtype=np.float32)
    W = np.asarray(W, dtype=np.float32)
    D = np.asarray(D_bernoulli, dtype=np.float32)

    # --- host: forward rfft of (x*D) blocks ---
    xd = (x * D[None, :]).reshape(B_TOTAL, K_IN, BLK)
    Xf = np.fft.rfft(xd, axis=-1)                 # [B, 32, 65]
    Xr = np.ascontiguousarray(Xf.real.transpose(2, 1, 0)).astype(np.float32)  # [65, 32, B]
    Xi = np.ascontiguousarray(Xf.imag.transpose(2, 1, 0)).astype(np.float32)
    XR = Xr[:64]                                  # [64 bins, 32 j, B]
    XI = Xi[:64].copy()
    XI[0] = Xr[64]                                # R64 rides in the I0 slot

    # fp8 e3m4 input scale; the weights absorb 1/s
    s = XSCALE_TGT / max(np.abs(XR).max(), np.abs(XI).max())
    XRq = (XR * s).astype(e3m4)
    XIq = (XI * s).astype(e3m4)

    # --- host: 64x64 bin matrices M2 = [[A,C],[B,D]] (lhsT) ---
    Vf = np.conj(np.fft.rfft(W, axis=-1))         # [o, j, 65]
    VR = np.ascontiguousarray(Vf.real.transpose(2, 1, 0)).astype(np.float32)  # [65, j, o]
    VI = np.ascontiguousarray(Vf.imag.transpose(2, 1, 0)).astype(np.float32)
    M2 = np.empty((64, 64, 64), dtype=np.float32)  # [bin, jc, oc]
    M2[:, :K_IN, :K_OUT] = VR[:64]                 # A  (YR += A.XR)
    M2[:, :K_IN, K_OUT:] = VI[:64]                 # C  (YI += C.XR)
    M2[:, K_IN:, :K_OUT] = -VI[:64]                # B  (YR += B.XI)
    M2[:, K_IN:, K_OUT:] = VR[:64]                 # D  (YI += D.XI)
    M2[0, :K_IN, K_OUT:] = 0.0                     # bin 0/64 are real-only
    M2[0, K_IN:, :K_OUT] = 0.0
    M2[0, K_IN:, K_OUT:] = VR[64]                  # R64 channel in the I0 slot

    # --- bin permutation: 32 lowest-energy bins -> fp8 output half ---
    XRf = XRq.astype(np.float32)
    XIf = XIq.astype(np.float32)
    PX = np.concatenate(
        [(XRf ** 2).mean(axis=2), (XIf ** 2).mean(axis=2)], axis=1
    )                                              # [64, 64] E[x2q^2] per jc
    proxy = np.einsum('fjo,fj->f', M2 ** 2, PX) / (s * s)
    wgt = np.full(64, 2.0); wgt[0] = 1.0
    order = np.argsort(proxy * wgt)
    lo_bins = np.sort(order[:32])                  # fp8 output half (half=1)
    hi_bins = np.sort(order[32:])                  # bf16 output half (half=0)
    slot_bin = np.empty((NG, 2, 2), dtype=np.int64)
    slot_bin[:, 0, :] = hi_bins.reshape(NG, 2)
    slot_bin[:, 1, :] = lo_bins.reshape(NG, 2)

    # --- per-bin output scales for the fp8 half (hard C-S bound) ---
    # |psum[oc]| <= ||(M2/s)[:,oc]|| * max_b ||x2q*s...|| ; rhs on device
    # is XRq/XIq (already scaled by s), lhsT is M2/s
    x2n = np.sqrt(
        (XRf ** 2).sum(axis=1) + (XIf ** 2).sum(axis=1)
    ).max(axis=1)                                  # [64] max_b ||x2q_b||
    coln = np.sqrt((M2 ** 2).sum(axis=1)).max(axis=1) / s   # [64] max_oc ||.||
    bound = coln * x2n                             # per-bin hard |psum| bound
    sigma = np.ones(64, dtype=np.float32)
    sigma[lo_bins] = YSCALE_TGT / bound[lo_bins]

    # --- weights -> wt[p, g, u, oc] per the slot map, scaled ---
    M2s = M2 * (sigma / s)[:, None, None]
    wt_host = np.empty((128, NG, 2, 64), dtype=bf16)
    for g in range(NG):
        for half in range(2):
            for u in range(2):
                wt_host[64 * half : 64 * half + 64, g, u, :] = (
                    M2s[slot_bin[g, half, u]].astype(bf16)
                )

    # --- pack inputs per the slot map: xq[un, 64h+32c+j, gi, u, m] ---
    Z = np.stack([XRq, XIq], axis=1)               # [64, 2(comp), 32, B]
    xq_all = np.empty((NU, 2, 2, K_IN, 4, 2, B_TOTAL), dtype=e3m4)
    # axes: [un, half, comp, j, gi, u, m]
    for g in range(NG):
        for half in range(2):
            for u in range(2):
                xq_all[g // 4, half, :, :, g % 4, u, :] = Z[slot_bin[g, half, u]]
    xq_all = xq_all.reshape(NU, 128, 4, 2, B_TOTAL)

    in_maps = []
    for c in range(N_CORES):
        sl = slice(c * B_SHARD, (c + 1) * B_SHARD)
        in_maps.append(
            {"xq": np.ascontiguousarray(xq_all[:, :, :, :, sl]), "wt": wt_host}
        )

    nc = _get_module()
    res = run_bass_kernel_spmd(nc, in_maps, core_ids=list(range(N_CORES)))

    # --- host: unpack spectra, irfft, reassemble ---
    inv_sigma = (1.0 / sigma).astype(np.float32)
    out = np.empty((B_TOTAL, D_OUT), dtype=np.float32)
    NB = BLK // 2 + 1
    for c in range(N_CORES):
        PS = np.empty((64, 2, K_OUT, B_SHARD), dtype=np.float32)  # [bin, comp, o, m]
        for half, key in ((0, "ybf"), (1, "yq8")):
            y = np.asarray(res.results[c][key], dtype=np.float32)  # [NPO,128,2,2,512]
            # y[pr, 64*oh + 32*comp + o, gj, mc, m'] ; bin slot (g=2pr+gj, half, u=oh)
            yb = y.reshape(NPO, 2, 2, K_OUT, 2, 2 * MM_FREE)  # [pr, oh, comp, o, gj, m]
            for pr in range(NPO):
                for gj in range(2):
                    for oh in range(2):
                        b = slot_bin[2 * pr + gj, half, oh]
                        PS[b] = yb[pr, oh, :, :, gj, :] * inv_sigma[b]
        psR, psI = PS[:, 0], PS[:, 1]                    # [64, o, m]
        Yf = np.zeros((B_SHARD, K_OUT, NB), dtype=np.complex64)
        Yf[:, :, :64] = (psR + 1j * psI).transpose(2, 1, 0)
        Yf[:, :, 0] = psR[0].T
        Yf[:, :, 64] = psI[0].T
        ob = np.fft.irfft(Yf, n=BLK, axis=-1)            # [m, 32, 128]
        out[c * B_SHARD : (c + 1) * B_SHARD] = ob.reshape(B_SHARD, D_OUT)
    return out


# revision 10
# speedup vs baseline: 1.4297x; 1.0877x over previous
"""BlockCirculantLinear kernel for 8x TRN2 NeuronCores — FFT-domain einsum, v3.

Math: out = (x*D) @ M with M block-circulant (32x32 grid of 128-circulants).
Host does the cheap O(B d log b) rfft/irfft + packing; the device does the
frequency-domain einsum out_f = X_f @ V_f (a 32x32 complex matmul per bin).

Device kernel (per core, 1/8 of the batch = 1024 rows):
- Each bin's complex matmul is ONE dense 64x64 real matmul via the
  [[Re, Im], [-Im, Re]] block form: rhs = [XR; XI] (64 partitions),
  out = [YR; YI]. Four bins run concurrently on the four 64x64 quadrants
  of the PE array via tile_position; 128 MMs total.
- Input spectra ship as fp8 E3M4 (4 mantissa bits), scaled by 14/max
  on host; mixed-dtype matmul (bf16 lhsT x fp8e3 rhs) keeps weights
  full precision. Input HBM: 8MB -> 4.2MB per core.
- Output: the 32 lowest-energy bins (ranked by a per-bin energy proxy)
  are routed to the partition-half-1 slots and evacuated as fp8 E3M4
  (psum -> SBUF cast on the scalar engine); the 32 high-energy bins
  stay bf16 (vector engine). Per-bin scales, folded into the weights
  with a hard Cauchy-Schwarz bound (|psum| <= 12.9 < 15.5 max normal,
  overflow impossible), are divided back out on the host. Output HBM:
  8MB -> 6.3MB per core. End-to-end rel err ~1.6e-2 (gate 2e-2;
  all-bf16 is 3.1e-3, and all-fp8-out would be 1.92e-2 - too tight).
- PSUM is organized as 2-bank tiles [128, 2, 512] so each group needs
  one [128,1024] evacuation copy per engine (the v2 per-bank copies at
  ~0.7us each made the copy pipeline the critical path).
- DMA: strict FIFO on the Sync/SP HWDGE ring only: weights, 4x 1MB
  input units, then the output DMAs — reads get absolute priority,
  which is the makespan-optimal schedule at the ~360GB/s per-NC HBM
  cap. (v2 put the weights on the ACT ring, which started ~4us late
  and stalled the first real matmuls to 17.7us, HAM-cold.) Total
  ~11MB/core -> ~31us DMA window + ~9us NRT preamble + ~8us postamble.
- HAM pre-warm garbage MMs bridge until the first input unit lands.
"""

import numpy as np
import ml_dtypes

B_TOTAL = 8192
D_IN = 4096
D_OUT = 4096
BLK = 128
K_IN = D_IN // BLK    # 32
K_OUT = D_OUT // BLK  # 32
N_CORES = 8
B_SHARD = B_TOTAL // N_CORES  # 1024
NG = 16               # groups of 4 bins (64 plane-pairs)
NU = 8                # input DMA units (2 groups = 0.5MB each)
NPO = 8               # output DMA pairs (2 groups each)
MM_FREE = 512         # moving free dim per matmul (one PSUM bank)
XSCALE_TGT = 14.0     # fp8 e3m4 max normal is 15.5
YSCALE_TGT = 12.9

_compiled = None


def _build_module():
    import concourse.bass as bass
    import concourse.tile as tile
    from concourse import bacc, mybir

    nc = bacc.Bacc("TRN2", target_bir_lowering=False, debug=False)

    bf = mybir.dt.bfloat16
    f8 = mybir.dt.float8e3
    f32 = mybir.dt.float32

    # xq[unit, p, gi, u, m]: p = 64*half + 32*comp + j; slot = (g, half, u)
    xq = nc.dram_tensor("xq", [NU, 128, 2, 2, B_SHARD], f8, kind="ExternalInput")
    # wt[p, g, u, oc]: p = 64*half + jc; lhsT of the 64x64 bin matrix
    wt = nc.dram_tensor("wt", [128, NG, 2, 64], bf, kind="ExternalInput")
    # ybf[pair, p, gj, mc, m']: bank A (input-half-0 bins), bf16
    ybf = nc.dram_tensor("ybf", [NPO, 128, 2, 2, MM_FREE], bf, kind="ExternalOutput")
    # yq8[pair, p, gj, mc, m']: bank B (input-half-1 bins), fp8 e3m4
    yq8 = nc.dram_tensor("yq8", [NPO, 128, 2, 2, MM_FREE], f8, kind="ExternalOutput")

    U_ELEMS = 128 * 2 * 2 * B_SHARD   # 0.5MB fp8 per input unit
    PO_ELEMS = 128 * 2 * 2 * MM_FREE  # per output pair per tensor

    with tile.TileContext(nc) as tc:
        with (
            tc.tile_pool(name="sb", bufs=1) as spool,
            tc.tile_pool(name="psum", bufs=2, space="PSUM") as ppool,
        ):
            w = spool.tile([128, NG, 2, 64], bf, name="wt")
            # weights lead the Sync ring: land ~10.2us, before the first
            # real MMs need them (~12.6) — the ACT ring starts ~4us late
            nc.sync.dma_start(w[:], wt[:])

            scratch = spool.tile([128, MM_FREE], bf, name="scratch")
            nc.vector.memset(scratch[:], 0.0)

            xts = []
            for ui in range(NU):
                xt = spool.tile(
                    [128, 2, 2, B_SHARD], f8, tag="xt", name=f"xt{ui}", bufs=NU
                )
                nc.sync.dma_start(
                    xt[:],
                    bass.AP(xq, ui * U_ELEMS, [[2 * 2 * B_SHARD, 128], [1, 2 * 2 * B_SHARD]]),
                )
                xts.append(xt)

            otA = otB = None
            for g in range(NG):
                ui, gi = g // 2, g % 2
                xt = xts[ui]
                # psA: bank-pair for input-half-0 bins (a: cols 0-63, b: 64-127)
                # psB: bank-pair for input-half-1 bins (c, d); [128, mc, 512]
                psA = ppool.tile([128, 2, MM_FREE], f32, tag="psA", name=f"psA_{g}")
                psB = ppool.tile([128, 2, MM_FREE], f32, tag="psB", name=f"psB_{g}")
                if g == 0:
                    # HAM pre-warm: garbage matmuls while the weights and the
                    # first input unit stream in (~3.4us of sustained PE
                    # activity trips the 2.4GHz un-throttle); results are
                    # wiped by the real MMs' start=True
                    for k in range(14):
                        nc.tensor.matmul(
                            (psA if k % 2 == 0 else psB)[:, k % 2, :],
                            lhsT=scratch[:, 0:128],
                            rhs=scratch[:],
                            start=True,
                            stop=True,
                        )
                elif g % 2 != 0 and g < NG - 2:
                    # keep-warm filler; insurance against input-DMA jitter
                    nc.tensor.matmul(
                        psA[:, 0, :],
                        lhsT=scratch[:, 0:128],
                        rhs=scratch[:],
                        start=True,
                        stop=True,
                    )
                for mc in range(2):
                    s = slice(mc * MM_FREE, (mc + 1) * MM_FREE)
                    for half, u, ps, tp in (
                        (0, 0, psA, (0, 0)),
                        (0, 1, psA, (0, 64)),
                        (1, 0, psB, (64, 0)),
                        (1, 1, psB, (64, 64)),
                    ):
                        rows = slice(64 * half, 64 * half + 64)
                        cols = slice(tp[1], tp[1] + 64)
                        nc.tensor.matmul(
                            ps[cols, mc, :],
                            lhsT=w[rows, g, u, :],
                            rhs=xt[rows, gi, u, s],
                            start=True,
                            stop=True,
                            tile_position=tp,
                        )

                pr, gj = g // 2, g % 2
                if gj == 0:
                    otA = spool.tile([128, 2, 2, MM_FREE], bf, tag="otA", name=f"otA{pr}", bufs=NPO)
                    otB = spool.tile([128, 2, 2, MM_FREE], f8, tag="otB", name=f"otB{pr}", bufs=NPO)
                oA, oB = otA[:, gj], otB[:, gj]
                # evacuate PSUM: one [128,1024] copy per engine per group,
                # alternating the bank->engine map so DVE (1.19us/copy) and
                # ACT (1.0us) each carry a balanced mix — the copy pipeline
                # is the end-game critical path, not the DMA bytes
                if g % 2 == 0:
                    nc.vector.tensor_copy(oA, psA[:])
                    nc.scalar.copy(oB, psB[:])
                else:
                    nc.scalar.copy(oA, psA[:])
                    nc.vector.tensor_copy(oB, psB[:])

                if gj == 1:
                    nc.sync.dma_start(
                        bass.AP(ybf, pr * PO_ELEMS, [[2048, 128], [1, 2048]]),
                        otA[:],
                    )
                    nc.sync.dma_start(
                        bass.AP(yq8, pr * PO_ELEMS, [[2048, 128], [1, 2048]]),
                        otB[:],
                    )

    nc.compile()
    return nc


def _get_module():
    global _compiled
    if _compiled is None:
        _compiled = _build_module()
    return _compiled


def kernel(x: np.ndarray, W: np.ndarray, D_bernoulli: np.ndarray) -> np.ndarray:
    from concourse.bass_utils import run_bass_kernel_spmd

    bf16 = ml_dtypes.bfloat16
    e3m4 = ml_dtypes.float8_e3m4
    x = np.asarray(x, dtype=np.float32)
    W = np.asarray(W, dtype=np.float32)
    D = np.asarray(D_bernoulli, dtype=np.float32)

    # --- host: forward rfft of (x*D) blocks ---
    xd = (x * D[None, :]).reshape(B_TOTAL, K_IN, BLK)
    Xf = np.fft.rfft(xd, axis=-1)                 # [B, 32, 65]
    Xr = np.ascontiguousarray(Xf.real.transpose(2, 1, 0)).astype(np.float32)  # [65, 32, B]
    Xi = np.ascontiguousarray(Xf.imag.transpose(2, 1, 0)).astype(np.float32)
    XR = Xr[:64]                                  # [64 bins, 32 j, B]
    XI = Xi[:64].copy()
    XI[0] = Xr[64]                                # R64 rides in the I0 slot

    # fp8 e3m4 input scale; the weights absorb 1/s
    s = XSCALE_TGT / max(np.abs(XR).max(), np.abs(XI).max())
    XRq = (XR * s).astype(e3m4)
    XIq = (XI * s).astype(e3m4)

    # --- host: 64x64 bin matrices M2 = [[A,C],[B,D]] (lhsT) ---
    Vf = np.conj(np.fft.rfft(W, axis=-1))         # [o, j, 65]
    VR = np.ascontiguousarray(Vf.real.transpose(2, 1, 0)).astype(np.float32)  # [65, j, o]
    VI = np.ascontiguousarray(Vf.imag.transpose(2, 1, 0)).astype(np.float32)
    M2 = np.empty((64, 64, 64), dtype=np.float32)  # [bin, jc, oc]
    M2[:, :K_IN, :K_OUT] = VR[:64]                 # A  (YR += A.XR)
    M2[:, :K_IN, K_OUT:] = VI[:64]                 # C  (YI += C.XR)
    M2[:, K_IN:, :K_OUT] = -VI[:64]                # B  (YR += B.XI)
    M2[:, K_IN:, K_OUT:] = VR[:64]                 # D  (YI += D.XI)
    M2[0, :K_IN, K_OUT:] = 0.0                     # bin 0/64 are real-only
    M2[0, K_IN:, :K_OUT] = 0.0
    M2[0, K_IN:, K_OUT:] = VR[64]                  # R64 channel in the I0 slot

    # --- bin permutation: 32 lowest-energy bins -> fp8 output half ---
    XRf = XRq.astype(np.float32)
    XIf = XIq.astype(np.float32)
    PX = np.concatenate(
        [(XRf ** 2).mean(axis=2), (XIf ** 2).mean(axis=2)], axis=1
    )                                              # [64, 64] E[x2q^2] per jc
    proxy = np.einsum('fjo,fj->f', M2 ** 2, PX) / (s * s)
    wgt = np.full(64, 2.0); wgt[0] = 1.0
    order = np.argsort(proxy * wgt)
    lo_bins = np.sort(order[:32])                  # fp8 output half (half=1)
    hi_bins = np.sort(order[32:])                  # bf16 output half (half=0)
    slot_bin = np.empty((NG, 2, 2), dtype=np.int64)
    slot_bin[:, 0, :] = hi_bins.reshape(NG, 2)
    slot_bin[:, 1, :] = lo_bins.reshape(NG, 2)

    # --- per-bin output scales for the fp8 half (hard C-S bound) ---
    # |psum[oc]| <= ||(M2/s)[:,oc]|| * max_b ||x2q*s...|| ; rhs on device
    # is XRq/XIq (already scaled by s), lhsT is M2/s
    x2n = np.sqrt(
        (XRf ** 2).sum(axis=1) + (XIf ** 2).sum(axis=1)
    ).max(axis=1)                                  # [64] max_b ||x2q_b||
    coln = np.sqrt((M2 ** 2).sum(axis=1)).max(axis=1) / s   # [64] max_oc ||.||
    bound = coln * x2n                             # per-bin hard |psum| bound
    sigma = np.ones(64, dtype=np.float32)
    sigma[lo_bins] = YSCALE_TGT / bound[lo_bins]

    # --- weights -> wt[p, g, u, oc] per the slot map, scaled ---
    M2s = M2 * (sigma / s)[:, None, None]
    wt_host = np.empty((128, NG, 2, 64), dtype=bf16)
    for g in range(NG):
        for half in range(2):
            for u in range(2):
                wt_host[64 * half : 64 * half + 64, g, u, :] = (
                    M2s[slot_bin[g, half, u]].astype(bf16)
                )

    # --- pack inputs per the slot map: xq[un, 64h+32c+j, gi, u, m] ---
    Z = np.stack([XRq, XIq], axis=1)               # [64, 2(comp), 32, B]
    xq_all = np.empty((NU, 2, 2, K_IN, 2, 2, B_TOTAL), dtype=e3m4)
    # axes: [un, half, comp, j, gi, u, m]
    for g in range(NG):
        for half in range(2):
            for u in range(2):
                xq_all[g // 2, half, :, :, g % 2, u, :] = Z[slot_bin[g, half, u]]
    xq_all = xq_all.reshape(NU, 128, 2, 2, B_TOTAL)

    in_maps = []
    for c in range(N_CORES):
        sl = slice(c * B_SHARD, (c + 1) * B_SHARD)
        in_maps.append(
            {"xq": np.ascontiguousarray(xq_all[:, :, :, :, sl]), "wt": wt_host}
        )

    nc = _get_module()
    res = run_bass_kernel_spmd(nc, in_maps, core_ids=list(range(N_CORES)))

    # --- host: unpack spectra, irfft, reassemble ---
    inv_sigma = (1.0 / sigma).astype(np.float32)
    out = np.empty((B_TOTAL, D_OUT), dtype=np.float32)
    NB = BLK // 2 + 1
    for c in range(N_CORES):
        PS = np.empty((64, 2, K_OUT, B_SHARD), dtype=np.float32)  # [bin, comp, o, m]
        for half, key in ((0, "ybf"), (1, "yq8")):
            y = np.asarray(res.results[c][key], dtype=np.float32)  # [NPO,128,2,2,512]
            # y[pr, 64*oh + 32*comp + o, gj, mc, m'] ; bin slot (g=2pr+gj, half, u=oh)
            yb = y.reshape(NPO, 2, 2, K_OUT, 2, 2 * MM_FREE)  # [pr, oh, comp, o, gj, m]
            for pr in range(NPO):
                for gj in range(2):
                    for oh in range(2):
                        b = slot_bin[2 * pr + gj, half, oh]
                        PS[b] = yb[pr, oh, :, :, gj, :] * inv_sigma[b]
        psR, psI = PS[:, 0], PS[:, 1]                    # [64, o, m]
        Yf = np.zeros((B_SHARD, K_OUT, NB), dtype=np.complex64)
        Yf[:, :, :64] = (psR + 1j * psI).transpose(2, 1, 0)
        Yf[:, :, 0] = psR[0].T
        Yf[:, :, 64] = psI[0].T
        ob = np.fft.irfft(Yf, n=BLK, axis=-1)            # [m, 32, 128]
        out[c * B_SHARD : (c + 1) * B_SHARD] = ob.reshape(B_SHARD, D_OUT)
    return out
